# revision 105
# baseline (speedup 1.0000x reference)
"""Trainium2 Bass kernel for ComplementaryChannelInteraction.

Pipeline (per sample):
  1x1 conv (+folded BN1) -> ReLU -> channel attention softmax(-x@xT) ->
  3x3 conv (+folded BN2) -> ReLU -> global avg pool -> FC -> ReLU -> FC

Sharding: pure data parallel, B=128 -> 16 samples on each of 8 cores,
all params replicated.

Precision: conv1x1, x@xT and the attention-apply in bf16 (f32 PSUM);
3x3 conv in fp8 e4m3 DoubleRow (2x PE throughput).  fp8
weight-quantization error is neutralized by a mean-shift split
y = c + y': c is a per-channel constant computed ON THE HOST from
sample 0's attention output (c is an arbitrary shift, it only needs to
be close to the typical spatial mean of y), and corr = conv3(c-field)
is evaluated on the host with the EXACT folded f32 weights, shipped as
an input, and added into each group's PSUM by the vector engine.  The
fp8 weight error then only couples to the small zero-centered residual
y'.  Measured end-to-end ~8e-3 rel err (gate 2e-2).

conv3 layout: the pair's two y images live in ONE flat 31x16
zero-padded image (row stride 16, middle pad row shared), so every 3x3
tap is one contiguous 462-column stream and both samples share a
single DoubleRow weight load.  Garbage PSUM columns are never read.

Transposes: all x^T and E^T transposes go through the DMA XBAR
(SBUF->SBUF dma_start(transpose=True), 16x128 tiles) instead of PE
identity matmuls + PSUM drains.  E^T is ONE whole-[128,2048] transpose
per sample with a 3D output AP (ET[dp, ci, dj, c'] = E[128ci+c',
128dj+dp]); x^T is one transpose per (sample, m) from an m-split xcm
layout so the xxt moving operand stays contiguous.

Schedule: pair-phase-major with the previous pair's conv3 groups
interleaved as PE filler between attention phases, one conv1 chunk of
the NEXT pair pre-issued before each pair's y phase (covers the ET
XBAR latency), features prefetched two pairs ahead, and the fc1
contraction chasing the final pair's conv3 drains.  fc2 output is
computed sample-major (stationary = h, bias folded in as a ones-row of
the contraction) so the final DMA is 16 contiguous 800B rows.

CAUTION: the tile framework's DMA-sem pool tolerates only ~10 DMA
instructions per pair; one more splits/dispatches and multi-us
sem-recycle stalls appear on the sync queue.  Keep all steady-state
DMAs on nc.sync and batch maximally before changing any DMA counts.
"""
import contextlib
import ctypes
import sys
import types

import numpy as np
import ml_dtypes

import concourse.bass as bass
import concourse.tile as tile
import concourse.mybir as mybir
from concourse import bacc
from concourse.bass_utils import run_bass_kernel_spmd

dt = mybir.dt
F32, BF16, FP8 = dt.float32, dt.bfloat16, dt.float8e4
AF = mybir.ActivationFunctionType
ALU = mybir.AluOpType
AX = mybir.AxisListType
DR = mybir.MatmulPerfMode.DoubleRow

N_CORES = 8
B, CIN, C, H, W, NCOUT = 128, 2048, 512, 14, 14, 200
P = H * W            # 196
BPC = B // N_CORES   # 16 samples per core
KC = CIN // 128      # 16 contraction chunks for conv1
CC = C // 128        # 4 channel chunks
PCH = [(0, 128), (128, 68)]  # spatial chunks of 196: (offset, size)
EPS = 1e-5
SW = 64.0            # fp8 scale for w3
SY = 32.0            # fp8 scale for y'
DESCALE = 1.0 / (SW * SY)
STREAM = 462         # conv3 stream: both samples share the middle pad row
                     # (31 rows x 16 flat image; y of (v,h) at row 1+15v+h,
                     # output col q = 240v + 16h + x, input pos q + toff)
VOFF = 240           # per-sample output column offset in the stream

# ---------------------------------------------------------------- compat shims


def _install_drain_patch():
    """walrus here allows only ONE sync-wait per Drain; split the Tile
    kernel-tail drain into a chain of single-wait drains."""

    def _split_drain_and_barrier(self, tick_clock, wait_clock):
        from concourse.tile import ScopedClock

        drain_inst = self.nc.sync.drain()
        wait_clock.add_sem_waits(
            drain_inst.ins, ScopedClock({None: tick_clock.global_clock})
        )
        si = drain_inst.ins.sync_info
        waits = list(si.on_wait) if si is not None else []
        if len(waits) > 1:
            drain_inst.ins.sync_info = mybir.SyncInfo(
                on_wait=waits[:1], on_update=list(si.on_update)
            )
            for i in range(1, len(waits)):
                extra = self.nc.sync.drain()
                extra.ins.sync_info = mybir.SyncInfo(
                    on_wait=waits[i : i + 1], on_update=[]
                )
        self.nc.all_engine_barrier()
        assert self.sems is not None
        popped = self.nc._tile_sem_poison_stack.pop()
        assert popped is self._sem_poison
        self.nc.clear_and_free_semaphores(list(self.sems.allocated().values()))
        self.nc.all_engine_barrier()

    tile.TileContext._drain_and_barrier = _split_drain_and_barrier


def _install_ntff_hook(so_path="/opt/axon/libaxon_pjrt.so"):
    """antenv.axon_hooks is missing in this image; recreate it so
    trace=True (NTFF profiling) works instead of crashing on import."""
    if "antenv.axon_hooks" in sys.modules:
        return
    mod = types.ModuleType("antenv.axon_hooks")
    state = {"hook": None}
    mod.set_axon_ntff_profile_hook = lambda h: state.__setitem__("hook", h)
    mod.get_axon_ntff_profile_hook = lambda: state["hook"]
    sys.modules["antenv.axon_hooks"] = mod
    try:
        import antenv

        antenv.axon_hooks = mod
    except ImportError:
        pass
    try:
        lib = ctypes.CDLL(so_path)
        if not hasattr(lib, "axon_start_nrt_profile"):
            return
        lib.axon_start_nrt_profile.argtypes = [
            ctypes.POINTER(ctypes.c_int64),
            ctypes.c_size_t,
        ]
        lib.axon_start_nrt_profile.restype = ctypes.c_int64
        lib.axon_stop_nrt_profile.argtypes = [ctypes.c_char_p]
        lib.axon_stop_nrt_profile.restype = ctypes.c_int64
    except OSError:
        return

    @contextlib.contextmanager
    def _hook(output_dir, device_ids):
        import jax

        jax.devices()
        if device_ids:
            ids = (ctypes.c_int64 * len(device_ids))(*device_ids)
            rc = lib.axon_start_nrt_profile(ids, len(device_ids))
        else:
            rc = lib.axon_start_nrt_profile(None, 0)
        if rc != 0:
            raise RuntimeError(f"axon_start_nrt_profile rc={rc}")
        try:
            yield
        finally:
            n = lib.axon_stop_nrt_profile(str(output_dir).encode())
            if n < 0:
                raise RuntimeError(f"axon_stop_nrt_profile rc={n}")
            print(f"profile: {n} file(s) written to {output_dir}", file=sys.stderr)

    state["hook"] = _hook


def install_shims():
    _install_drain_patch()
    _install_ntff_hook()


# ---------------------------------------------------------------- bass program


def build_program(n_samples=BPC):
    install_shims()
    nc = bacc.Bacc(
        "TRN2", target_bir_lowering=False, debug=False, num_devices=N_CORES
    )

    feat_d = nc.dram_tensor("feat", [n_samples, CIN, P], BF16, kind="ExternalInput")
    wpT_d = nc.dram_tensor("wpT", [CIN, C], BF16, kind="ExternalInput")
    t1c_d = nc.dram_tensor("t1c", [128, CC], F32, kind="ExternalInput")
    w3q_d = nc.dram_tensor("w3q", [128, 2, 2, 9, C], FP8, kind="ExternalInput")
    corr_d = nc.dram_tensor("corr", [128, CC, 512], BF16, kind="ExternalInput")
    c32_d = nc.dram_tensor("c32", [128, CC], F32, kind="ExternalInput")
    t2_d = nc.dram_tensor("t2", [CC, 128], F32, kind="ExternalInput")
    fc1_d = nc.dram_tensor("fc1", [CC, 128, NCOUT], F32, kind="ExternalInput")
    fc1b_d = nc.dram_tensor("fc1b", [2, 100], F32, kind="ExternalInput")
    fc2_d = nc.dram_tensor("fc2", [2, 101, NCOUT], F32, kind="ExternalInput")
    identb_d = nc.dram_tensor("identb", [128, 128], BF16, kind="ExternalInput")
    out_d = nc.dram_tensor("out", [n_samples, NCOUT], F32, kind="ExternalOutput")

    with tile.TileContext(nc) as tc:
        with contextlib.ExitStack() as ctx:
            wpool = ctx.enter_context(tc.tile_pool(name="weights", bufs=1))
            featp = ctx.enter_context(tc.tile_pool(name="featp", bufs=3))
            xcmp = ctx.enter_context(tc.tile_pool(name="xcmp", bufs=4))
            xcm2p = ctx.enter_context(tc.tile_pool(name="xcm2p", bufs=4))
            fTp = ctx.enter_context(tc.tile_pool(name="fTp", bufs=3))
            Ep = ctx.enter_context(tc.tile_pool(name="Ep", bufs=2))
            smallp = ctx.enter_context(tc.tile_pool(name="smallp", bufs=3))
            zscrp = ctx.enter_context(tc.tile_pool(name="zscrp", bufs=2))
            ps_c1 = ctx.enter_context(tc.tile_pool(name="ps_c1", bufs=2, space="PSUM"))
            ps_xxt = ctx.enter_context(tc.tile_pool(name="ps_xxt", bufs=2, space="PSUM"))
            ps_sm = ctx.enter_context(tc.tile_pool(name="ps_sm", bufs=2, space="PSUM"))
            ps_z = ctx.enter_context(tc.tile_pool(name="ps_z", bufs=2, space="PSUM"))

            # ---- feature loads: one [128, KC, 2, P] tile per pair.  Pair 0
            # stays chunked per-k (interleaved with wpT) so the first conv1
            # matmul only waits for a fraction of the startup DMA; later
            # pairs use 4 fused DMAs (4 k-chunks each) to keep the sync
            # queue's ~650ns-per-dispatch cost off the critical path while
            # still spreading the transfer over 4 DMA engines.
            def load_pair(g, eng=None, ways=1):
                # 4 instructions (4 DMA engines); DMA stays on the sync
                # queue (scalar for the pair-1 startup load: sync's 32
                # startup dispatches would delay it ~21us) and the per-pair
                # DMA count is kept low so the tile framework's small
                # DMA-sem pool never recycles a sem whose previous DMA is
                # still in flight.
                eng = eng or nc.sync
                ft = featp.tile([128, KC, 2, P], BF16, tag="feat")
                kq = KC // ways
                for q in range(ways):
                    for s in range(2):
                        eng.dma_start(
                            ft[:, kq * q : kq * (q + 1), s, :],
                            feat_d[2 * g + s, 128 * kq * q : 128 * kq * (q + 1), :]
                            .rearrange("(k c) p -> c k p", c=128),
                        )
                return ft

            wpT = []
            feat0 = featp.tile([128, KC, 2, P], BF16, tag="feat")
            early_feat1 = None
            for k in range(KC):
                wt = wpool.tile([128, C], BF16, tag=f"wpT{k}")
                nc.sync.dma_start(wt[:], wpT_d[k * 128 : (k + 1) * 128, :])
                wpT.append(wt)
                nc.sync.dma_start(
                    feat0[:, k, :, :],
                    feat_d[0:2, k * 128 : (k + 1) * 128, :].rearrange("s c p -> c s p"),
                )

            t1c = wpool.tile([128, CC], F32, tag="t1c")
            nc.sync.dma_start(t1c[:], t1c_d[:])
            identb = wpool.tile([128, 128], BF16, tag="identb")
            nc.sync.dma_start(identb[:], identb_d[:])
            if n_samples // 2 > 1:
                early_feat1 = load_pair(1, ways=4)

            # ---- bulk param loads on the gpsimd SWDGE queue: it has no
            # other work, so these dispatch immediately and land during
            # pair-0 compute without bursting through the sync queue's
            # DMA-sem pool (which would stall pair-1's fT transpose).
            # w3q split 12 ways: a single 2.36MB DMA rides ONE DMA engine
            # at ~22.5GB/s = 105us and pair-0 conv3 would stall.  A dummy
            # copy gated on the last wpT chunk delays the burst until the
            # startup-critical wpT/feat0/feat1 loads have the HBM to
            # themselves (w3q is first needed by pair-0 conv3 at ~47us).
            gate = wpool.tile([128, 1], BF16, tag="gate")
            nc.gpsimd.tensor_copy(gate[:], wpT[KC - 1][:, 0:1])
            w3q = wpool.tile([128, 2, 2, 9, C], FP8, tag="w3q")
            for jp in range(2):
                for jj in range(2):
                    for tg in range(3):
                        nc.gpsimd.dma_start(
                            w3q[:, jp, jj, 3 * tg : 3 * tg + 3, :],
                            w3q_d[:, jp, jj, 3 * tg : 3 * tg + 3, :],
                        )
            state = {"w3q": w3q}
            # corr row (host-computed exact conv3 of the c-field) + c
            corr_sb = wpool.tile([128, CC, 512], BF16, tag="corr")
            nc.gpsimd.dma_start(corr_sb[:], corr_d[:])
            c32 = wpool.tile([128, CC], F32, tag="c32")
            nc.gpsimd.dma_start(c32[:], c32_d[:])
            t2sb = wpool.tile([128, CC], F32, tag="t2sb")
            nc.gpsimd.dma_start(t2sb[:], t2_d[:].rearrange("j p -> p j"))
            state["t2sb"] = t2sb
            fc1sb = wpool.tile([128, CC, NCOUT], F32, tag="fc1sb")
            nc.gpsimd.dma_start(fc1sb[:], fc1_d[:].rearrange("j p o -> p j o"))
            state["fc1sb"] = fc1sb
            fc1bsb = wpool.tile([128, 2], F32, tag="fc1bsb")
            nc.gpsimd.dma_start(fc1bsb[:100, :], fc1b_d[:].rearrange("m p -> p m"))
            state["fc1bsb"] = fc1bsb
            fc2sb = wpool.tile([128, 2, NCOUT], F32, tag="fc2sb")
            nc.gpsimd.dma_start(fc2sb[:101, :, :], fc2_d[:].rearrange("m p o -> p m o"))
            state["fc2sb"] = fc2sb

            # persistent fp8 padded-y tiles: [p, jp, jj, 496], a flat 31x16
            # zero-padded image holding BOTH samples of a pair (rows 1-14 =
            # s0, row 15 shared zero pad, rows 16-29 = s1); borders stay 0.
            ypads = []
            for par in range(2):
                yp = wpool.tile([128, 2, 2, 496], FP8, tag=f"ypad{par}")
                nc.vector.memset(yp[:], 0.0)
                ypads.append(yp)
            # accumulated pooled z for the whole per-core batch
            zall = wpool.tile([128, CC, n_samples], F32, tag="zall")

            def emit_conv3_group(gi, i):
                """One output-channel chunk of the 3x3 conv for pair gi:
                fp8 DoubleRow + exact-shift correction, both samples in one
                478-wide stream.  Emitted between attention phases of the
                NEXT pair so the PE never idles while attention drains."""
                ypad = ypads[gi % 2]
                w3q = state["w3q"]
                pz = ps_z.tile([128, 512], F32, tag="z")
                for tap in range(9):
                    toff = (tap // 3) * 16 + (tap % 3)
                    for jp in range(2):
                        nc.tensor.matmul(
                            pz[:, 0:STREAM],
                            w3q[:, jp, :, tap, i * 128 : (i + 1) * 128],
                            ypad[:, jp, :, toff : toff + STREAM],
                            start=(tap == 0 and jp == 0),
                            stop=(tap == 8 and jp == 1),
                            perf_mode=DR,
                        )
                # corr row added by the vector engine so the PE doesn't
                # spend a 478-wide identity matmul on it.
                nc.vector.tensor_tensor(
                    out=pz[:, 0:STREAM],
                    in0=pz[:, 0:STREAM],
                    in1=corr_sb[:, i, 0:STREAM],
                    op=ALU.add,
                )
                for v in range(2):
                    s = 2 * gi + v
                    zscr = zscrp.tile([128, H, W], BF16, tag="zscr")
                    nc.scalar.activation(
                        zscr[:],
                        pz[:, VOFF * v : VOFF * v + 224]
                        .rearrange("p (h x) -> p h x", x=16)[:, :, 0:14],
                        AF.Relu,
                        bias=state["t2sb"][:, i : i + 1],
                        scale=DESCALE,
                        accum_out=zall[:, i, s : s + 1],
                    )

            n_pairs = n_samples // 2
            # ---- conv1 chunk emitter.  xcm layout [c, m, i, np]
            # (n = 128m+np, m-major split of the spatial dim): the fT XBAR
            # input ([:, m, :, :]) and the xxt moving operand
            # ([:pn, m, :, :]) are then both fully contiguous -- a strided
            # moving AP costs ~220ns extra per xxt matmul.  m=1 tail
            # (np 68..127) is dead space.
            def emit_conv1_chunk(feat, xcm, i):
                pc = ps_c1.tile([128, 2, P], F32, tag="c1")
                for k in range(KC):
                    nc.tensor.matmul(
                        pc[:],
                        wpT[k][:, i * 128 : (i + 1) * 128],
                        feat[:, k, :, :],
                        start=(k == 0),
                        stop=(k == KC - 1),
                    )
                for v in range(2):
                    nc.scalar.activation(
                        xcm[v][:, 0, i, :], pc[:, v, 0:128], AF.Relu,
                        bias=t1c[:, i : i + 1],
                    )
                    # 68-col tail drained by the vector engine so the
                    # split doesn't double the scalar queue's load
                    # (relu(x + b) = max(x + b, 0) via tensor_scalar).
                    nc.vector.tensor_scalar(
                        out=xcm[v][:, 1, i, 0:68],
                        in0=pc[:, v, 128:196],
                        scalar1=t1c[:, i : i + 1],
                        scalar2=0.0,
                        op0=ALU.add,
                        op1=ALU.max,
                    )

            def alloc_xcm():
                return [xcmp.tile([128, 2, CC, 128], BF16, tag="xcm",
                                  name=f"xcm{next(_xcm_ctr)}")
                        for _ in range(2)]

            _xcm_ctr = iter(range(1000))

            prefetched = {}
            xcm_of = {}
            pre_done = {}
            for gi in range(n_pairs):
                ypad = ypads[gi % 2]
                feat = feat0 if gi == 0 else prefetched.pop(gi)
                # conv3 groups of the previous pair, interleaved as PE filler
                filler = (
                    [lambda i=i: emit_conv3_group(gi - 1, i) for i in range(CC)]
                    if gi > 0 else [lambda: None] * CC
                )

                # ---- 1x1 conv; chunks 0..1 may already have been emitted
                # as pre-work during the previous pair (software pipelining:
                # they give the PE cross-pair work while this pair's serial
                # drain->XBAR->exp chain runs).
                xcm = xcm_of.pop(gi, None) or alloc_xcm()
                for i in range(pre_done.pop(gi, 0), CC):
                    emit_conv1_chunk(feat, xcm, i)

                # j-major copy of xcm for the y-apply moving operand: lets
                # y stream 196 contiguous cols instead of 256 (the m-split
                # layout xbar needs has the m=1 junk tail mid-stream).
                # One DVE copy per sample, hidden under the xxt phase.
                xcm2 = []
                for v in range(2):
                    x2 = xcm2p.tile([128, CC, 2, 128], BF16, tag="xcm2",
                                    name=f"xcm2_{next(_xcm_ctr)}")
                    nc.vector.tensor_copy(
                        x2[:], xcm[v][:, :, :, :].rearrange("p m j n -> p j m n")
                    )
                    xcm2.append(x2)

                # ---- transpose to spatial-major fT via the DMA XBAR: ONE
                # SBUF->SBUF transpose instruction per sample (the ~1.2us
                # dispatch cost on the sync queue is fixed per instruction,
                # so batch all 8 [128,128] blocks into it).  Layout
                # fT2[np, i, m, c'] = x[128i+c', 128m+np].
                fTs = []
                for v in range(2):
                    fT_sb = fTp.tile([128, 2, CC, 128], BF16, tag="fT")
                    fTs.append(fT_sb)
                    for m in range(2):
                        nc.sync.dma_start(
                            fT_sb[:, m, :, :],
                            xcm[v][:, m, :, :],
                            transpose=True,
                        )

                # prefetch features two pairs ahead: pair gi+1's features
                # must be resident for its conv1 pre-work emitted later in
                # THIS pair (4 DMAs x 400KB need ~18us each).
                if gi == 0 and early_feat1 is not None:
                    prefetched[1] = early_feat1
                if gi + 2 < n_pairs:
                    prefetched[gi + 2] = load_pair(gi + 2)

                filler[0]()

                # ---- xxt + softmax numerator E per sample, with conv3
                # filler between the two samples.  Row-shifted:
                # E[c,d] = exp(-xxt[c,d]+m_c) (required: some rows have
                # min xxt > 87; exp(-xxt) would underflow to a zero row).
                Es = []
                ETs = []
                zi32s = []
                for v in range(2):
                    fT_sb = fTs[v]
                    E_sb = Ep.tile([128, CC, C], BF16, tag="E")
                    Es.append(E_sb)
                    ET_sb = Ep.tile([128, CC, CC, 128], BF16, tag="ET")
                    ETs.append(ET_sb)
                    zrow = smallp.tile([128, CC], F32, tag="zrow")
                    zinv = smallp.tile([128, CC], F32, tag="zinv")
                    zinv32 = smallp.tile([128, CC], F32, tag="zinv32")
                    zi32s.append(zinv32)
                    mrow = smallp.tile([128, CC], F32, tag="mrow")
                    for i in range(CC):
                        pxxt = ps_xxt.tile([128, C], F32, tag="xxt")
                        for m, (po, pn) in enumerate(PCH):
                            nc.tensor.matmul(
                                pxxt[:],
                                fT_sb[:pn, m, i, :],
                                fT_sb[:pn, m, :, :],
                                start=(m == 0),
                                stop=(m == 1),
                            )
                        nc.vector.tensor_reduce(
                            out=mrow[:, i : i + 1], in_=pxxt[:], op=ALU.min, axis=AX.X
                        )
                        nc.scalar.activation(
                            E_sb[:, i, :],
                            pxxt[:],
                            AF.Exp,
                            bias=mrow[:, i : i + 1],
                            scale=-1.0,
                            accum_out=zrow[:, i : i + 1],
                        )
                    # ONE XBAR transpose of the whole E.  NOTE: splitting
                    # this in two -- even for a single pair -- perturbs the
                    # global DMA-sem round-robin and re-triggers multi-us
                    # sem-recycling stalls on the sync queue (the pool
                    # tolerates ~10 DMA instructions per pair; more is over
                    # the cliff).
                    nc.sync.dma_start(
                        ET_sb[:, :, :, :],
                        E_sb[:, :, :],
                        transpose=True,
                    )
                    nc.vector.reciprocal(zinv[:], zrow[:])
                    nc.vector.tensor_scalar_mul(zinv32[:], zinv[:], SY)
                    filler[1 + v]()
                    # pre-work: ONE conv1 chunk of the NEXT pair right
                    # before the y phase, sized to absorb the ~4us the PE
                    # would otherwise idle waiting for the ET XBAR (more
                    # pre-work overshoots and delays y's dependents).  The
                    # last pair has no next conv1; move its G3 filler here
                    # instead (its y-v0 then never waits on filler).
                    if v == 1:
                        if gi + 1 < n_pairs:
                            xcm_of[gi + 1] = alloc_xcm()
                            emit_conv1_chunk(
                                prefetched[gi + 1], xcm_of[gi + 1], 0
                            )
                            pre_done[gi + 1] = 1
                        else:
                            filler[3]()

                # ---- y = (E @ x)/Z per sample (ET came from the XBAR);
                # conv3 filler between the samples.
                for v in range(2):
                    s = 2 * gi + v
                    zinv32 = zi32s[v]
                    ET_sb = ETs[v]
                    for i in range(CC):
                        # 256-wide psum: moving is the m-split xcm, so
                        # output cols q=128m+np coincide with n for q<196;
                        # cols 196..255 are garbage and never read.
                        py = ps_sm.tile([128, 256], F32, tag="small")
                        for j in range(CC):
                            nc.tensor.matmul(
                                py[:, 0:P],
                                ET_sb[:, i, j, :],
                                xcm2[v][:, j, :, :]
                                .rearrange("p m n -> p (m n)")[:, 0:P],
                                start=(j == 0),
                                stop=(j == CC - 1),
                            )
                        nc.vector.tensor_scalar(
                            out=ypad[:, i // 2, i % 2, :]
                            .rearrange("p (r c) -> p r c", c=16)
                            [:, 1 + 15 * v : 15 + 15 * v, 1:15],
                            in0=py[:, 0:P].rearrange("p (h w) -> p h w", h=H),
                            scalar1=zinv32[:, i : i + 1],
                            scalar2=c32[:, i : i + 1],
                            op0=ALU.mult,
                            op1=ALU.subtract,
                        )
                    if v == 0 and gi + 1 < n_pairs:
                        filler[3]()

            # ---- final pair's conv3 groups, with the fc1 j-contraction
            # steps chasing each group's drains (fc1 j needs zall[:, j, :]
            # complete, i.e. group j of the last pair drained -- interleaving
            # turns the tail's drain wait into overlapped PE work).
            phs = [ps_sm.tile([128, n_samples], F32, tag="small",
                              name=f"ph{m}") for m in range(2)]
            for i in range(CC):
                emit_conv3_group(n_pairs - 1, i)
                for m in range(2):
                    nc.tensor.matmul(
                        phs[m][:100, :],
                        state["fc1sb"][:, i, m * 100 : (m + 1) * 100],
                        zall[:, i, :],
                        start=(i == 0),
                        stop=(i == CC - 1),
                    )

            # ---- FC head over the whole per-core batch.  fc1 stays
            # o-major; fc2 is computed sample-major (stationary = h, with a
            # ones-row at partition 100 folding in fc2_b) so the output PSUM
            # is [n_samples, 200] and the final DMA is 16 contiguous rows.
            h_sb = smallp.tile([128, 2, n_samples], F32, tag="h")
            # ones-row at partition 100 (engine partition offsets must be
            # multiples of 32: memset 96:128 first, ReLU then overwrites
            # 96:100 -- WAW order is preserved by the tile framework).
            nc.vector.memset(h_sb[96:128, :, :], 1.0)
            for m in range(2):
                nc.scalar.activation(
                    h_sb[:100, m, :], phs[m][:100, :], AF.Relu,
                    bias=state["fc1bsb"][:100, m : m + 1],
                )
            sfT = smallp.tile([128, NCOUT], F32, tag="sfT")
            psf = ps_sm.tile([128, NCOUT], F32, tag="small")
            for m in range(2):
                nc.tensor.matmul(
                    psf[:n_samples, :],
                    h_sb[:101, m, :],
                    state["fc2sb"][:101, m, :],
                    start=(m == 0),
                    stop=(m == 1),
                )
            nc.scalar.copy(sfT[:n_samples, :], psf[:n_samples, :])
            nc.sync.dma_start(out_d[:, :], sfT[:n_samples, :])

    nc.compile()
    return nc


# ---------------------------------------------------------------- host wrapper

_prog_cache = {}


def _get_program(n_samples=BPC):
    if n_samples not in _prog_cache:
        _prog_cache[n_samples] = build_program(n_samples)
    return _prog_cache[n_samples]


def _cast8(x):
    return np.clip(x, -240.0, 240.0).astype(ml_dtypes.float8_e4m3)


def prepare_host_inputs(inputs):
    """Fold BN into weights, build the per-core replicated param arrays."""
    s1 = inputs["bn1_gamma"] / np.sqrt(inputs["bn1_var"] + EPS)
    t1 = (inputs["b_reduce"] - inputs["bn1_mean"]) * s1 + inputs["bn1_beta"]
    Wp = inputs["w_reduce"].reshape(C, CIN) * s1[:, None]
    wpT = np.ascontiguousarray(Wp.T).astype(ml_dtypes.bfloat16)  # [2048, 512]
    t1c = np.ascontiguousarray(t1.reshape(CC, 128).T)            # [128, CC]

    s2 = inputs["bn2_gamma"] / np.sqrt(inputs["bn2_var"] + EPS)
    t2 = (inputs["b3"] - inputs["bn2_mean"]) * s2 + inputs["bn2_beta"]
    w3p = inputs["w3"] * s2[:, None, None, None]            # [co, ci, ky, kx]
    w3s = (w3p * SW).astype(np.float32)
    w3q8 = _cast8(w3s)
    # [co, ci, tap] -> [ci, tap, co] -> [jp, jj, p, tap, co] -> [p, jp, jj, tap, co]
    w3q_l = np.ascontiguousarray(
        w3q8.reshape(C, C, 9).transpose(1, 2, 0).reshape(2, 2, 128, 9, C)
        .transpose(2, 0, 1, 3, 4)
    )
    t2_a = np.ascontiguousarray(t2.reshape(CC, 128))

    # ---- host-side shift vector c and exact correction row.
    # c is an arbitrary per-channel shift (it only needs to be close to the
    # typical spatial mean of y so the fp8-quantized residual y' = SY*y/Z - c
    # stays small); compute it from sample 0's attention output in f32.
    # corr = conv3(c-field) with the EXACT (pre-quantization) folded weights
    # at the same SW*SY scale the device PSUM carries.
    f0 = np.asarray(inputs["feature"], np.float32)[0].reshape(CIN, P)
    x0 = np.maximum(Wp.astype(np.float32) @ f0 + t1[:, None], 0.0)
    xxt0 = x0 @ x0.T
    mr = xxt0.min(axis=1)
    E0 = np.exp(mr[:, None] - xxt0)
    y0 = (E0 @ x0) / E0.sum(axis=1)[:, None]
    c_dev = (SY * y0.mean(axis=1)).astype(np.float32)       # [C], SY scale
    # u[co, tap] = sum_ci w3s[co, ci, tap] * c_dev[ci]
    u = np.einsum("oit,i->ot", w3s.reshape(C, C, 9), c_dev)
    corr14 = np.zeros((C, H, W), np.float32)
    for tap in range(9):
        dy, dx = tap // 3 - 1, tap % 3 - 1
        ys0, ys1 = max(0, -dy), min(H, H - dy)
        xs0, xs1 = max(0, -dx), min(W, W - dx)
        corr14[:, ys0:ys1, xs0:xs1] += u[:, tap, None, None]
    # pack into the conv3 stream layout: col q = 240v + 16h + x
    corr_l = np.zeros((128, CC, 512), np.float32)
    for i in range(CC):
        blk = np.pad(corr14[i * 128 : (i + 1) * 128],
                     ((0, 0), (0, 0), (0, 2)))                # [128, 14, 16]
        for voff in (0, 240):
            corr_l[:, i, voff : voff + 224] = blk.reshape(128, 224)
    corr_l = corr_l.astype(ml_dtypes.bfloat16)
    c32_l = np.ascontiguousarray(c_dev.reshape(CC, 128).T)   # [128, CC]

    fc1p = (inputs["fc1_w"] / float(P)).astype(np.float32)  # fold 1/196 mean
    fc1 = np.ascontiguousarray(fc1p.T.reshape(CC, 128, NCOUT))
    fc1b = np.ascontiguousarray(inputs["fc1_b"].reshape(2, 100))
    # fc2 with the bias folded in as contraction row 100 of the m=0 chunk
    # (the on-chip stationary h gets a matching ones-row).
    fc2 = np.zeros((2, 101, NCOUT), np.float32)
    fc2[:, :100, :] = inputs["fc2_w"].T.reshape(2, 100, NCOUT)
    fc2[0, 100, :] = inputs["fc2_b"]
    return {
        "wpT": wpT,
        "t1c": t1c,
        "w3q": w3q_l,
        "corr": corr_l,
        "c32": c32_l,
        "t2": t2_a,
        "fc1": fc1,
        "fc1b": fc1b,
        "fc2": fc2,
        "identb": np.eye(128, dtype=ml_dtypes.bfloat16),
    }


def run(inputs, n_samples=BPC, n_cores=N_CORES, trace=False):
    nc = _get_program(n_samples)
    params = prepare_host_inputs(inputs)
    feat = np.asarray(inputs["feature"], np.float32).reshape(B, CIN, P).astype(ml_dtypes.bfloat16)
    in_maps = []
    for c in range(n_cores):
        m = dict(params)
        m["feat"] = np.ascontiguousarray(feat[c * n_samples : (c + 1) * n_samples])
        in_maps.append(m)
    res = run_bass_kernel_spmd(nc, in_maps, list(range(n_cores)), trace=trace)
    out = np.concatenate([res.results[c]["out"] for c in range(n_cores)], axis=0)
    return out, res


def kernel(**inputs):
    inputs = {k: np.asarray(v) for k, v in inputs.items()}
    out, _ = run(inputs)
    return out.astype(np.float32)



# revision 106
# speedup vs baseline: 1.0104x; 1.0104x over previous
"""Trainium2 Bass kernel for ComplementaryChannelInteraction.

Pipeline (per sample):
  1x1 conv (+folded BN1) -> ReLU -> channel attention softmax(-x@xT) ->
  3x3 conv (+folded BN2) -> ReLU -> global avg pool -> FC -> ReLU -> FC

Sharding: pure data parallel, B=128 -> 16 samples on each of 8 cores,
all params replicated.

Precision: conv1x1, x@xT and the attention-apply in bf16 (f32 PSUM);
3x3 conv in fp8 e4m3 DoubleRow (2x PE throughput).  fp8
weight-quantization error is neutralized by a mean-shift split
y = c + y': c is a per-channel constant computed ON THE HOST from
sample 0's attention output (c is an arbitrary shift, it only needs to
be close to the typical spatial mean of y), and corr = conv3(c-field)
is evaluated on the host with the EXACT folded f32 weights, shipped as
an input, and added into each group's PSUM by the vector engine.  The
fp8 weight error then only couples to the small zero-centered residual
y'.  Measured end-to-end ~8e-3 rel err (gate 2e-2).

conv3 layout: the pair's two y images live in ONE flat 31x16
zero-padded image (row stride 16, middle pad row shared), so every 3x3
tap is one contiguous 462-column stream and both samples share a
single DoubleRow weight load.  Garbage PSUM columns are never read.

Transposes: all x^T and E^T transposes go through the DMA XBAR
(SBUF->SBUF dma_start(transpose=True), 16x128 tiles) instead of PE
identity matmuls + PSUM drains.  E^T is ONE whole-[128,2048] transpose
per sample with a 3D output AP (ET[dp, ci, dj, c'] = E[128ci+c',
128dj+dp]); x^T is one transpose per (sample, m) from an m-split xcm
layout so the xxt moving operand stays contiguous.

Schedule: pair-phase-major with the previous pair's conv3 groups
interleaved as PE filler between attention phases, one conv1 chunk of
the NEXT pair pre-issued before each pair's y phase (covers the ET
XBAR latency), features prefetched two pairs ahead, and the fc1
contraction chasing the final pair's conv3 drains.  fc2 output is
computed sample-major (stationary = h, bias folded in as a ones-row of
the contraction) so the final DMA is 16 contiguous 800B rows.

CAUTION: the tile framework's DMA-sem pool tolerates only ~10 DMA
instructions per pair; one more splits/dispatches and multi-us
sem-recycle stalls appear on the sync queue.  Keep all steady-state
DMAs on nc.sync and batch maximally before changing any DMA counts.
"""
import contextlib
import ctypes
import sys
import types

import numpy as np
import ml_dtypes

import concourse.bass as bass
import concourse.tile as tile
import concourse.mybir as mybir
from concourse import bacc
from concourse.bass_utils import run_bass_kernel_spmd

dt = mybir.dt
F32, BF16, FP8 = dt.float32, dt.bfloat16, dt.float8e4
AF = mybir.ActivationFunctionType
ALU = mybir.AluOpType
AX = mybir.AxisListType
DR = mybir.MatmulPerfMode.DoubleRow

N_CORES = 8
B, CIN, C, H, W, NCOUT = 128, 2048, 512, 14, 14, 200
P = H * W            # 196
BPC = B // N_CORES   # 16 samples per core
KC = CIN // 128      # 16 contraction chunks for conv1
CC = C // 128        # 4 channel chunks
PCH = [(0, 128), (128, 68)]  # spatial chunks of 196: (offset, size)
EPS = 1e-5
SW = 64.0            # fp8 scale for w3
SY = 32.0            # fp8 scale for y'
DESCALE = 1.0 / (SW * SY)
STREAM = 462         # conv3 stream: both samples share the middle pad row
                     # (31 rows x 16 flat image; y of (v,h) at row 1+15v+h,
                     # output col q = 240v + 16h + x, input pos q + toff)
VOFF = 240           # per-sample output column offset in the stream

# ---------------------------------------------------------------- compat shims


def _install_drain_patch():
    """walrus here allows only ONE sync-wait per Drain; split the Tile
    kernel-tail drain into a chain of single-wait drains."""

    def _split_drain_and_barrier(self, tick_clock, wait_clock):
        from concourse.tile import ScopedClock

        drain_inst = self.nc.sync.drain()
        wait_clock.add_sem_waits(
            drain_inst.ins, ScopedClock({None: tick_clock.global_clock})
        )
        si = drain_inst.ins.sync_info
        waits = list(si.on_wait) if si is not None else []
        if len(waits) > 1:
            drain_inst.ins.sync_info = mybir.SyncInfo(
                on_wait=waits[:1], on_update=list(si.on_update)
            )
            for i in range(1, len(waits)):
                extra = self.nc.sync.drain()
                extra.ins.sync_info = mybir.SyncInfo(
                    on_wait=waits[i : i + 1], on_update=[]
                )
        self.nc.all_engine_barrier()
        assert self.sems is not None
        popped = self.nc._tile_sem_poison_stack.pop()
        assert popped is self._sem_poison
        self.nc.clear_and_free_semaphores(list(self.sems.allocated().values()))
        self.nc.all_engine_barrier()

    tile.TileContext._drain_and_barrier = _split_drain_and_barrier


def _install_ntff_hook(so_path="/opt/axon/libaxon_pjrt.so"):
    """antenv.axon_hooks is missing in this image; recreate it so
    trace=True (NTFF profiling) works instead of crashing on import."""
    if "antenv.axon_hooks" in sys.modules:
        return
    mod = types.ModuleType("antenv.axon_hooks")
    state = {"hook": None}
    mod.set_axon_ntff_profile_hook = lambda h: state.__setitem__("hook", h)
    mod.get_axon_ntff_profile_hook = lambda: state["hook"]
    sys.modules["antenv.axon_hooks"] = mod
    try:
        import antenv

        antenv.axon_hooks = mod
    except ImportError:
        pass
    try:
        lib = ctypes.CDLL(so_path)
        if not hasattr(lib, "axon_start_nrt_profile"):
            return
        lib.axon_start_nrt_profile.argtypes = [
            ctypes.POINTER(ctypes.c_int64),
            ctypes.c_size_t,
        ]
        lib.axon_start_nrt_profile.restype = ctypes.c_int64
        lib.axon_stop_nrt_profile.argtypes = [ctypes.c_char_p]
        lib.axon_stop_nrt_profile.restype = ctypes.c_int64
    except OSError:
        return

    @contextlib.contextmanager
    def _hook(output_dir, device_ids):
        import jax

        jax.devices()
        if device_ids:
            ids = (ctypes.c_int64 * len(device_ids))(*device_ids)
            rc = lib.axon_start_nrt_profile(ids, len(device_ids))
        else:
            rc = lib.axon_start_nrt_profile(None, 0)
        if rc != 0:
            raise RuntimeError(f"axon_start_nrt_profile rc={rc}")
        try:
            yield
        finally:
            n = lib.axon_stop_nrt_profile(str(output_dir).encode())
            if n < 0:
                raise RuntimeError(f"axon_stop_nrt_profile rc={n}")
            print(f"profile: {n} file(s) written to {output_dir}", file=sys.stderr)

    state["hook"] = _hook


def install_shims():
    _install_drain_patch()
    _install_ntff_hook()


# ---------------------------------------------------------------- bass program


def build_program(n_samples=BPC):
    install_shims()
    nc = bacc.Bacc(
        "TRN2", target_bir_lowering=False, debug=False, num_devices=N_CORES
    )

    feat_d = nc.dram_tensor("feat", [n_samples, CIN, P], BF16, kind="ExternalInput")
    wpT_d = nc.dram_tensor("wpT", [CIN, C], BF16, kind="ExternalInput")
    t1c_d = nc.dram_tensor("t1c", [128, CC], F32, kind="ExternalInput")
    w3q_d = nc.dram_tensor("w3q", [128, 2, 2, 9, C], FP8, kind="ExternalInput")
    corr_d = nc.dram_tensor("corr", [128, CC, 512], BF16, kind="ExternalInput")
    c32_d = nc.dram_tensor("c32", [128, CC], F32, kind="ExternalInput")
    t2_d = nc.dram_tensor("t2", [CC, 128], F32, kind="ExternalInput")
    fc1_d = nc.dram_tensor("fc1", [CC, 128, NCOUT], F32, kind="ExternalInput")
    fc1b_d = nc.dram_tensor("fc1b", [2, 100], F32, kind="ExternalInput")
    fc2_d = nc.dram_tensor("fc2", [2, 101, NCOUT], F32, kind="ExternalInput")
    identb_d = nc.dram_tensor("identb", [128, 128], BF16, kind="ExternalInput")
    out_d = nc.dram_tensor("out", [n_samples, NCOUT], F32, kind="ExternalOutput")

    with tile.TileContext(nc) as tc:
        with contextlib.ExitStack() as ctx:
            wpool = ctx.enter_context(tc.tile_pool(name="weights", bufs=1))
            featp = ctx.enter_context(tc.tile_pool(name="featp", bufs=3))
            xcmp = ctx.enter_context(tc.tile_pool(name="xcmp", bufs=4))
            xcm2p = ctx.enter_context(tc.tile_pool(name="xcm2p", bufs=4))
            fTp = ctx.enter_context(tc.tile_pool(name="fTp", bufs=3))
            Ep = ctx.enter_context(tc.tile_pool(name="Ep", bufs=2))
            smallp = ctx.enter_context(tc.tile_pool(name="smallp", bufs=3))
            zscrp = ctx.enter_context(tc.tile_pool(name="zscrp", bufs=2))
            ps_c1 = ctx.enter_context(tc.tile_pool(name="ps_c1", bufs=2, space="PSUM"))
            ps_xxt = ctx.enter_context(tc.tile_pool(name="ps_xxt", bufs=2, space="PSUM"))
            ps_sm = ctx.enter_context(tc.tile_pool(name="ps_sm", bufs=2, space="PSUM"))
            ps_z = ctx.enter_context(tc.tile_pool(name="ps_z", bufs=2, space="PSUM"))

            # ---- feature loads: one [128, KC, 2, P] tile per pair.  Pair 0
            # stays chunked per-k (interleaved with wpT) so the first conv1
            # matmul only waits for a fraction of the startup DMA; later
            # pairs use 4 fused DMAs (4 k-chunks each) to keep the sync
            # queue's ~650ns-per-dispatch cost off the critical path while
            # still spreading the transfer over 4 DMA engines.
            def load_pair(g, eng=None, ways=1):
                # 4 instructions (4 DMA engines); DMA stays on the sync
                # queue (scalar for the pair-1 startup load: sync's 32
                # startup dispatches would delay it ~21us) and the per-pair
                # DMA count is kept low so the tile framework's small
                # DMA-sem pool never recycles a sem whose previous DMA is
                # still in flight.
                eng = eng or nc.sync
                ft = featp.tile([128, KC, 2, P], BF16, tag="feat")
                kq = KC // ways
                for q in range(ways):
                    for s in range(2):
                        eng.dma_start(
                            ft[:, kq * q : kq * (q + 1), s, :],
                            feat_d[2 * g + s, 128 * kq * q : 128 * kq * (q + 1), :]
                            .rearrange("(k c) p -> c k p", c=128),
                        )
                return ft

            wpT = []
            feat0 = featp.tile([128, KC, 2, P], BF16, tag="feat")
            early_feat1 = None
            for k in range(KC):
                wt = wpool.tile([128, C], BF16, tag=f"wpT{k}")
                nc.sync.dma_start(wt[:], wpT_d[k * 128 : (k + 1) * 128, :])
                wpT.append(wt)
                # feat0 chunks on the gpsimd queue: startup dispatch is
                # serial per queue (~650ns each), so splitting wpT (sync)
                # and feat0 (gpsimd) across queues halves the ~21us of
                # dispatch ahead of the last chunk and pulls feat1 forward.
                nc.gpsimd.dma_start(
                    feat0[:, k, :, :],
                    feat_d[0:2, k * 128 : (k + 1) * 128, :].rearrange("s c p -> c s p"),
                )

            t1c = wpool.tile([128, CC], F32, tag="t1c")
            nc.sync.dma_start(t1c[:], t1c_d[:])
            identb = wpool.tile([128, 128], BF16, tag="identb")
            nc.sync.dma_start(identb[:], identb_d[:])
            if n_samples // 2 > 1:
                early_feat1 = load_pair(1, ways=4)

            # ---- bulk param loads on the gpsimd SWDGE queue: it has no
            # other work, so these dispatch immediately and land during
            # pair-0 compute without bursting through the sync queue's
            # DMA-sem pool (which would stall pair-1's fT transpose).
            # w3q split 12 ways: a single 2.36MB DMA rides ONE DMA engine
            # at ~22.5GB/s = 105us and pair-0 conv3 would stall.  A dummy
            # copy gated on the last wpT chunk delays the burst until the
            # startup-critical wpT/feat0/feat1 loads have the HBM to
            # themselves (w3q is first needed by pair-0 conv3 at ~47us).
            gate = wpool.tile([128, 1], BF16, tag="gate")
            nc.gpsimd.tensor_copy(gate[:], wpT[KC - 1][:, 0:1])
            w3q = wpool.tile([128, 2, 2, 9, C], FP8, tag="w3q")
            for jp in range(2):
                for jj in range(2):
                    for tg in range(3):
                        nc.gpsimd.dma_start(
                            w3q[:, jp, jj, 3 * tg : 3 * tg + 3, :],
                            w3q_d[:, jp, jj, 3 * tg : 3 * tg + 3, :],
                        )
            state = {"w3q": w3q}
            # corr row (host-computed exact conv3 of the c-field) + c
            corr_sb = wpool.tile([128, CC, 512], BF16, tag="corr")
            nc.gpsimd.dma_start(corr_sb[:], corr_d[:])
            c32 = wpool.tile([128, CC], F32, tag="c32")
            nc.gpsimd.dma_start(c32[:], c32_d[:])
            t2sb = wpool.tile([128, CC], F32, tag="t2sb")
            nc.gpsimd.dma_start(t2sb[:], t2_d[:].rearrange("j p -> p j"))
            state["t2sb"] = t2sb
            fc1sb = wpool.tile([128, CC, NCOUT], F32, tag="fc1sb")
            nc.gpsimd.dma_start(fc1sb[:], fc1_d[:].rearrange("j p o -> p j o"))
            state["fc1sb"] = fc1sb
            fc1bsb = wpool.tile([128, 2], F32, tag="fc1bsb")
            nc.gpsimd.dma_start(fc1bsb[:100, :], fc1b_d[:].rearrange("m p -> p m"))
            state["fc1bsb"] = fc1bsb
            fc2sb = wpool.tile([128, 2, NCOUT], F32, tag="fc2sb")
            nc.gpsimd.dma_start(fc2sb[:101, :, :], fc2_d[:].rearrange("m p o -> p m o"))
            state["fc2sb"] = fc2sb

            # persistent fp8 padded-y tiles: [p, jp, jj, 496], a flat 31x16
            # zero-padded image holding BOTH samples of a pair (rows 1-14 =
            # s0, row 15 shared zero pad, rows 16-29 = s1); borders stay 0.
            ypads = []
            for par in range(2):
                yp = wpool.tile([128, 2, 2, 496], FP8, tag=f"ypad{par}")
                nc.vector.memset(yp[:], 0.0)
                ypads.append(yp)
            # accumulated pooled z for the whole per-core batch
            zall = wpool.tile([128, CC, n_samples], F32, tag="zall")

            def emit_conv3_group(gi, i):
                """One output-channel chunk of the 3x3 conv for pair gi:
                fp8 DoubleRow + exact-shift correction, both samples in one
                478-wide stream.  Emitted between attention phases of the
                NEXT pair so the PE never idles while attention drains."""
                ypad = ypads[gi % 2]
                w3q = state["w3q"]
                pz = ps_z.tile([128, 512], F32, tag="z")
                for tap in range(9):
                    toff = (tap // 3) * 16 + (tap % 3)
                    for jp in range(2):
                        nc.tensor.matmul(
                            pz[:, 0:STREAM],
                            w3q[:, jp, :, tap, i * 128 : (i + 1) * 128],
                            ypad[:, jp, :, toff : toff + STREAM],
                            start=(tap == 0 and jp == 0),
                            stop=(tap == 8 and jp == 1),
                            perf_mode=DR,
                        )
                # corr row added by the vector engine so the PE doesn't
                # spend a 478-wide identity matmul on it.
                nc.vector.tensor_tensor(
                    out=pz[:, 0:STREAM],
                    in0=pz[:, 0:STREAM],
                    in1=corr_sb[:, i, 0:STREAM],
                    op=ALU.add,
                )
                for v in range(2):
                    s = 2 * gi + v
                    zscr = zscrp.tile([128, H, W], BF16, tag="zscr")
                    nc.scalar.activation(
                        zscr[:],
                        pz[:, VOFF * v : VOFF * v + 224]
                        .rearrange("p (h x) -> p h x", x=16)[:, :, 0:14],
                        AF.Relu,
                        bias=state["t2sb"][:, i : i + 1],
                        scale=DESCALE,
                        accum_out=zall[:, i, s : s + 1],
                    )

            n_pairs = n_samples // 2
            # ---- conv1 chunk emitter.  xcm layout [c, m, i, np]
            # (n = 128m+np, m-major split of the spatial dim): the fT XBAR
            # input ([:, m, :, :]) and the xxt moving operand
            # ([:pn, m, :, :]) are then both fully contiguous -- a strided
            # moving AP costs ~220ns extra per xxt matmul.  m=1 tail
            # (np 68..127) is dead space.
            def emit_conv1_chunk(feat, xcm, i):
                pc = ps_c1.tile([128, 2, P], F32, tag="c1")
                for k in range(KC):
                    nc.tensor.matmul(
                        pc[:],
                        wpT[k][:, i * 128 : (i + 1) * 128],
                        feat[:, k, :, :],
                        start=(k == 0),
                        stop=(k == KC - 1),
                    )
                for v in range(2):
                    nc.scalar.activation(
                        xcm[v][:, 0, i, :], pc[:, v, 0:128], AF.Relu,
                        bias=t1c[:, i : i + 1],
                    )
                    # 68-col tail drained by the vector engine so the
                    # split doesn't double the scalar queue's load
                    # (relu(x + b) = max(x + b, 0) via tensor_scalar).
                    nc.vector.tensor_scalar(
                        out=xcm[v][:, 1, i, 0:68],
                        in0=pc[:, v, 128:196],
                        scalar1=t1c[:, i : i + 1],
                        scalar2=0.0,
                        op0=ALU.add,
                        op1=ALU.max,
                    )

            def alloc_xcm():
                return [xcmp.tile([128, 2, CC, 128], BF16, tag="xcm",
                                  name=f"xcm{next(_xcm_ctr)}")
                        for _ in range(2)]

            _xcm_ctr = iter(range(1000))

            prefetched = {}
            xcm_of = {}
            pre_done = {}
            for gi in range(n_pairs):
                ypad = ypads[gi % 2]
                feat = feat0 if gi == 0 else prefetched.pop(gi)
                # conv3 groups of the previous pair, interleaved as PE filler
                filler = (
                    [lambda i=i: emit_conv3_group(gi - 1, i) for i in range(CC)]
                    if gi > 0 else [lambda: None] * CC
                )

                # ---- 1x1 conv; chunks 0..1 may already have been emitted
                # as pre-work during the previous pair (software pipelining:
                # they give the PE cross-pair work while this pair's serial
                # drain->XBAR->exp chain runs).
                xcm = xcm_of.pop(gi, None) or alloc_xcm()
                for i in range(pre_done.pop(gi, 0), CC):
                    emit_conv1_chunk(feat, xcm, i)

                # j-major copy of xcm for the y-apply moving operand: lets
                # y stream 196 contiguous cols instead of 256 (the m-split
                # layout xbar needs has the m=1 junk tail mid-stream).
                # One DVE copy per sample, hidden under the xxt phase.
                xcm2 = []
                for v in range(2):
                    x2 = xcm2p.tile([128, CC, 2, 128], BF16, tag="xcm2",
                                    name=f"xcm2_{next(_xcm_ctr)}")
                    nc.vector.tensor_copy(
                        x2[:], xcm[v][:, :, :, :].rearrange("p m j n -> p j m n")
                    )
                    xcm2.append(x2)

                # ---- transpose to spatial-major fT via the DMA XBAR: ONE
                # SBUF->SBUF transpose instruction per sample (the ~1.2us
                # dispatch cost on the sync queue is fixed per instruction,
                # so batch all 8 [128,128] blocks into it).  Layout
                # fT2[np, i, m, c'] = x[128i+c', 128m+np].
                fTs = []
                for v in range(2):
                    fT_sb = fTp.tile([128, 2, CC, 128], BF16, tag="fT")
                    fTs.append(fT_sb)
                    for m in range(2):
                        nc.sync.dma_start(
                            fT_sb[:, m, :, :],
                            xcm[v][:, m, :, :],
                            transpose=True,
                        )

                # prefetch features two pairs ahead: pair gi+1's features
                # must be resident for its conv1 pre-work emitted later in
                # THIS pair (4 DMAs x 400KB need ~18us each).
                if gi == 0 and early_feat1 is not None:
                    prefetched[1] = early_feat1
                if gi + 2 < n_pairs:
                    prefetched[gi + 2] = load_pair(gi + 2)

                filler[0]()

                # ---- xxt + softmax numerator E per sample, with conv3
                # filler between the two samples.  Row-shifted:
                # E[c,d] = exp(-xxt[c,d]+m_c) (required: some rows have
                # min xxt > 87; exp(-xxt) would underflow to a zero row).
                Es = []
                ETs = []
                zi32s = []
                for v in range(2):
                    fT_sb = fTs[v]
                    E_sb = Ep.tile([128, CC, C], BF16, tag="E")
                    Es.append(E_sb)
                    ET_sb = Ep.tile([128, CC, CC, 128], BF16, tag="ET")
                    ETs.append(ET_sb)
                    zrow = smallp.tile([128, CC], F32, tag="zrow")
                    zinv = smallp.tile([128, CC], F32, tag="zinv")
                    zinv32 = smallp.tile([128, CC], F32, tag="zinv32")
                    zi32s.append(zinv32)
                    mrow = smallp.tile([128, CC], F32, tag="mrow")
                    for i in range(CC):
                        pxxt = ps_xxt.tile([128, C], F32, tag="xxt")
                        for m, (po, pn) in enumerate(PCH):
                            nc.tensor.matmul(
                                pxxt[:],
                                fT_sb[:pn, m, i, :],
                                fT_sb[:pn, m, :, :],
                                start=(m == 0),
                                stop=(m == 1),
                            )
                        nc.vector.tensor_reduce(
                            out=mrow[:, i : i + 1], in_=pxxt[:], op=ALU.min, axis=AX.X
                        )
                        nc.scalar.activation(
                            E_sb[:, i, :],
                            pxxt[:],
                            AF.Exp,
                            bias=mrow[:, i : i + 1],
                            scale=-1.0,
                            accum_out=zrow[:, i : i + 1],
                        )
                    # ONE XBAR transpose of the whole E.  NOTE: splitting
                    # this in two -- even for a single pair -- perturbs the
                    # global DMA-sem round-robin and re-triggers multi-us
                    # sem-recycling stalls on the sync queue (the pool
                    # tolerates ~10 DMA instructions per pair; more is over
                    # the cliff).
                    nc.sync.dma_start(
                        ET_sb[:, :, :, :],
                        E_sb[:, :, :],
                        transpose=True,
                    )
                    nc.vector.reciprocal(zinv[:], zrow[:])
                    nc.vector.tensor_scalar_mul(zinv32[:], zinv[:], SY)
                    filler[1 + v]()
                    # pre-work: ONE conv1 chunk of the NEXT pair right
                    # before the y phase, sized to absorb the ~4us the PE
                    # would otherwise idle waiting for the ET XBAR (more
                    # pre-work overshoots and delays y's dependents).  The
                    # last pair has no next conv1; move its G3 filler here
                    # instead (its y-v0 then never waits on filler).
                    if v == 1:
                        if gi + 1 < n_pairs:
                            xcm_of[gi + 1] = alloc_xcm()
                            emit_conv1_chunk(
                                prefetched[gi + 1], xcm_of[gi + 1], 0
                            )
                            pre_done[gi + 1] = 1
                        else:
                            filler[3]()

                # ---- y = (E @ x)/Z per sample (ET came from the XBAR);
                # conv3 filler between the samples.
                for v in range(2):
                    s = 2 * gi + v
                    zinv32 = zi32s[v]
                    ET_sb = ETs[v]
                    for i in range(CC):
                        # 256-wide psum: moving is the m-split xcm, so
                        # output cols q=128m+np coincide with n for q<196;
                        # cols 196..255 are garbage and never read.
                        py = ps_sm.tile([128, 256], F32, tag="small")
                        for j in range(CC):
                            nc.tensor.matmul(
                                py[:, 0:P],
                                ET_sb[:, i, j, :],
                                xcm2[v][:, j, :, :]
                                .rearrange("p m n -> p (m n)")[:, 0:P],
                                start=(j == 0),
                                stop=(j == CC - 1),
                            )
                        nc.vector.tensor_scalar(
                            out=ypad[:, i // 2, i % 2, :]
                            .rearrange("p (r c) -> p r c", c=16)
                            [:, 1 + 15 * v : 15 + 15 * v, 1:15],
                            in0=py[:, 0:P].rearrange("p (h w) -> p h w", h=H),
                            scalar1=zinv32[:, i : i + 1],
                            scalar2=c32[:, i : i + 1],
                            op0=ALU.mult,
                            op1=ALU.subtract,
                        )
                    if v == 0 and gi + 1 < n_pairs:
                        filler[3]()

            # ---- final pair's conv3 groups, with the fc1 j-contraction
            # steps chasing each group's drains (fc1 j needs zall[:, j, :]
            # complete, i.e. group j of the last pair drained -- interleaving
            # turns the tail's drain wait into overlapped PE work).
            phs = [ps_sm.tile([128, n_samples], F32, tag="small",
                              name=f"ph{m}") for m in range(2)]
            for i in range(CC):
                emit_conv3_group(n_pairs - 1, i)
                for m in range(2):
                    nc.tensor.matmul(
                        phs[m][:100, :],
                        state["fc1sb"][:, i, m * 100 : (m + 1) * 100],
                        zall[:, i, :],
                        start=(i == 0),
                        stop=(i == CC - 1),
                    )

            # ---- FC head over the whole per-core batch.  fc1 stays
            # o-major; fc2 is computed sample-major (stationary = h, with a
            # ones-row at partition 100 folding in fc2_b) so the output PSUM
            # is [n_samples, 200] and the final DMA is 16 contiguous rows.
            h_sb = smallp.tile([128, 2, n_samples], F32, tag="h")
            # ones-row at partition 100 (engine partition offsets must be
            # multiples of 32: memset 96:128 first, ReLU then overwrites
            # 96:100 -- WAW order is preserved by the tile framework).
            nc.vector.memset(h_sb[96:128, :, :], 1.0)
            for m in range(2):
                nc.scalar.activation(
                    h_sb[:100, m, :], phs[m][:100, :], AF.Relu,
                    bias=state["fc1bsb"][:100, m : m + 1],
                )
            sfT = smallp.tile([128, NCOUT], F32, tag="sfT")
            psf = ps_sm.tile([128, NCOUT], F32, tag="small")
            for m in range(2):
                nc.tensor.matmul(
                    psf[:n_samples, :],
                    h_sb[:101, m, :],
                    state["fc2sb"][:101, m, :],
                    start=(m == 0),
                    stop=(m == 1),
                )
            nc.scalar.copy(sfT[:n_samples, :], psf[:n_samples, :])
            nc.sync.dma_start(out_d[:, :], sfT[:n_samples, :])

    nc.compile()
    return nc


# ---------------------------------------------------------------- host wrapper

_prog_cache = {}


def _get_program(n_samples=BPC):
    if n_samples not in _prog_cache:
        _prog_cache[n_samples] = build_program(n_samples)
    return _prog_cache[n_samples]


def _cast8(x):
    return np.clip(x, -240.0, 240.0).astype(ml_dtypes.float8_e4m3)


def prepare_host_inputs(inputs):
    """Fold BN into weights, build the per-core replicated param arrays."""
    s1 = inputs["bn1_gamma"] / np.sqrt(inputs["bn1_var"] + EPS)
    t1 = (inputs["b_reduce"] - inputs["bn1_mean"]) * s1 + inputs["bn1_beta"]
    Wp = inputs["w_reduce"].reshape(C, CIN) * s1[:, None]
    wpT = np.ascontiguousarray(Wp.T).astype(ml_dtypes.bfloat16)  # [2048, 512]
    t1c = np.ascontiguousarray(t1.reshape(CC, 128).T)            # [128, CC]

    s2 = inputs["bn2_gamma"] / np.sqrt(inputs["bn2_var"] + EPS)
    t2 = (inputs["b3"] - inputs["bn2_mean"]) * s2 + inputs["bn2_beta"]
    w3p = inputs["w3"] * s2[:, None, None, None]            # [co, ci, ky, kx]
    w3s = (w3p * SW).astype(np.float32)
    w3q8 = _cast8(w3s)
    # [co, ci, tap] -> [ci, tap, co] -> [jp, jj, p, tap, co] -> [p, jp, jj, tap, co]
    w3q_l = np.ascontiguousarray(
        w3q8.reshape(C, C, 9).transpose(1, 2, 0).reshape(2, 2, 128, 9, C)
        .transpose(2, 0, 1, 3, 4)
    )
    t2_a = np.ascontiguousarray(t2.reshape(CC, 128))

    # ---- host-side shift vector c and exact correction row.
    # c is an arbitrary per-channel shift (it only needs to be close to the
    # typical spatial mean of y so the fp8-quantized residual y' = SY*y/Z - c
    # stays small); compute it from sample 0's attention output in f32.
    # corr = conv3(c-field) with the EXACT (pre-quantization) folded weights
    # at the same SW*SY scale the device PSUM carries.
    f0 = np.asarray(inputs["feature"], np.float32)[0].reshape(CIN, P)
    x0 = np.maximum(Wp.astype(np.float32) @ f0 + t1[:, None], 0.0)
    xxt0 = x0 @ x0.T
    mr = xxt0.min(axis=1)
    E0 = np.exp(mr[:, None] - xxt0)
    y0 = (E0 @ x0) / E0.sum(axis=1)[:, None]
    c_dev = (SY * y0.mean(axis=1)).astype(np.float32)       # [C], SY scale
    # u[co, tap] = sum_ci w3s[co, ci, tap] * c_dev[ci]
    u = np.einsum("oit,i->ot", w3s.reshape(C, C, 9), c_dev)
    corr14 = np.zeros((C, H, W), np.float32)
    for tap in range(9):
        dy, dx = tap // 3 - 1, tap % 3 - 1
        ys0, ys1 = max(0, -dy), min(H, H - dy)
        xs0, xs1 = max(0, -dx), min(W, W - dx)
        corr14[:, ys0:ys1, xs0:xs1] += u[:, tap, None, None]
    # pack into the conv3 stream layout: col q = 240v + 16h + x
    corr_l = np.zeros((128, CC, 512), np.float32)
    for i in range(CC):
        blk = np.pad(corr14[i * 128 : (i + 1) * 128],
                     ((0, 0), (0, 0), (0, 2)))                # [128, 14, 16]
        for voff in (0, 240):
            corr_l[:, i, voff : voff + 224] = blk.reshape(128, 224)
    corr_l = corr_l.astype(ml_dtypes.bfloat16)
    c32_l = np.ascontiguousarray(c_dev.reshape(CC, 128).T)   # [128, CC]

    fc1p = (inputs["fc1_w"] / float(P)).astype(np.float32)  # fold 1/196 mean
    fc1 = np.ascontiguousarray(fc1p.T.reshape(CC, 128, NCOUT))
    fc1b = np.ascontiguousarray(inputs["fc1_b"].reshape(2, 100))
    # fc2 with the bias folded in as contraction row 100 of the m=0 chunk
    # (the on-chip stationary h gets a matching ones-row).
    fc2 = np.zeros((2, 101, NCOUT), np.float32)
    fc2[:, :100, :] = inputs["fc2_w"].T.reshape(2, 100, NCOUT)
    fc2[0, 100, :] = inputs["fc2_b"]
    return {
        "wpT": wpT,
        "t1c": t1c,
        "w3q": w3q_l,
        "corr": corr_l,
        "c32": c32_l,
        "t2": t2_a,
        "fc1": fc1,
        "fc1b": fc1b,
        "fc2": fc2,
        "identb": np.eye(128, dtype=ml_dtypes.bfloat16),
    }


def run(inputs, n_samples=BPC, n_cores=N_CORES, trace=False):
    nc = _get_program(n_samples)
    params = prepare_host_inputs(inputs)
    feat = np.asarray(inputs["feature"], np.float32).reshape(B, CIN, P).astype(ml_dtypes.bfloat16)
    in_maps = []
    for c in range(n_cores):
        m = dict(params)
        m["feat"] = np.ascontiguousarray(feat[c * n_samples : (c + 1) * n_samples])
        in_maps.append(m)
    res = run_bass_kernel_spmd(nc, in_maps, list(range(n_cores)), trace=trace)
    out = np.concatenate([res.results[c]["out"] for c in range(n_cores)], axis=0)
    return out, res


def kernel(**inputs):
    inputs = {k: np.asarray(v) for k, v in inputs.items()}
    out, _ = run(inputs)
    return out.astype(np.float32)

